# revision 31
# baseline (speedup 1.0000x reference)
"""Trainium2 Bass kernel for nn_BEVGNNModelLoaded (gnn_message_passing).

Data-parallel over batch dim B=32 across 8 NeuronCores (4 batches/core).

Math (per batch of N=8 node images):
  feats[n, s, c]   = conv_patch(img[n])            (stride-32 32x32 patches; S=49, C=384)
  a = feats @ W1a; g = feats @ W1b                 ([N, S, HM], HM=512)
  h[i, j, s, :]    = relu(a[i, s] + g[j, s] + b1)
  m[i, j]          = (sum_s h / S) @ W2 + b2       ([N, N, 256])
  heads            = m @ [Wp|Wpv|Wr|Wrv] + biases  ([N, N, 7])
  edge_out         = heads * radius_mask

Device-side structure:
  * Patch conv as matmul over K=3072 (24 chunks of 128). Because stride-32
    patches don't overlap, im2col is a pure permutation — done on the host
    when packing per-core inputs, so each batch's pixels arrive in ONE
    contiguous [128, 9408] DMA.
  * The four heads + W2 collapse: W2h = (W2 @ W_heads)/S, so the device
    computes pooled @ W2h directly ([512, 7]); m is never materialized.
  * The s-pooling folds into the final matmul accumulation (49 strided-rhs
    matmuls per HM chunk) — no reduce instruction at all.
Host side: bias2h add + mask multiply on the [2048, 7] result (trivial),
the radius mask itself, zeros for node_preds.

Compute dtype bf16 (inputs cast on host), fp32 PSUM accumulation.
"""
import numpy as np
import ml_dtypes
from contextlib import ExitStack

import concourse.bass as bass
import concourse.bacc as bacc
import concourse.tile as tile
from concourse import mybir
from concourse.bass_utils import run_bass_kernel_spmd

F32 = mybir.dt.float32
BF16 = mybir.dt.bfloat16
AX = mybir.AluOpType
AF = mybir.ActivationFunctionType

# problem constants (hardcoded per contract)
B, N, C_IN, H, W = 32, 8, 3, 224, 224
PATCH, C, S, HM, CO = 32, 384, 49, 512, 256
COMM_RANGE = 0.5
N_CORES = 8
B_LOC = B // N_CORES          # 4 batches per core
K = C_IN * PATCH * PATCH      # 3072 contraction
NKC = K // 128                # 24 K-chunks
NS = N * S                    # 392 free cols per batch
E = N * N                     # 64 edges per batch
NHC = HM // 128               # 4 HM chunks
NMC = C // 128                # 3 C chunks

HC_ORDER = (2, 3, 0, 1)   # ACT-relu'd chunks first, DVE-relu'd last

_CACHE = {}


def _build_nc():
    nc = bacc.Bacc(trn_type="TRN2", target_bir_lowering=False, debug=False)
    # img already im2col'd on host: [B_LOC, 128, NKC*NS]; [p, kc*392 + s*8 + n]
    img_d = nc.dram_tensor("img", [B_LOC, 128, NKC * NS], BF16, kind="ExternalInput").ap()
    wp_d = nc.dram_tensor("wp", [K, C], BF16, kind="ExternalInput").ap()
    w1a_d = nc.dram_tensor("w1a", [C, HM], BF16, kind="ExternalInput").ap()
    w1b_d = nc.dram_tensor("w1b", [C, HM], BF16, kind="ExternalInput").ap()
    w2t_d = nc.dram_tensor("w2t", [HM, 7], BF16, kind="ExternalInput").ap()
    bp_d = nc.dram_tensor("bpatch", [C], F32, kind="ExternalInput").ap()
    b1_d = nc.dram_tensor("b1", [HM], F32, kind="ExternalInput").ap()
    out_d = nc.dram_tensor("out", [B_LOC // 2, 7, 2 * E], F32, kind="ExternalOutput").ap()

    with tile.TileContext(nc) as tc, ExitStack() as ctx:
        wpool = ctx.enter_context(tc.tile_pool(name="wpool", bufs=1))
        rpool = ctx.enter_context(tc.tile_pool(name="rpool", bufs=2))
        fpool = ctx.enter_context(tc.tile_pool(name="fpool", bufs=2))
        agpool = ctx.enter_context(tc.tile_pool(name="agpool", bufs=2))
        tpool = ctx.enter_context(tc.tile_pool(name="tpool", bufs=2))
        opool = ctx.enter_context(tc.tile_pool(name="opool", bufs=2))
        psA = ctx.enter_context(tc.tile_pool(name="psA", bufs=6, space="PSUM"))
        psB = ctx.enter_context(tc.tile_pool(name="psB", bufs=2, space="PSUM"))

        # first pair's image DMAs go out BEFORE the weight DMAs so the PE
        # isn't stuck behind ~5MB of weight traffic at kernel start
        def emit_rows(pair):
            tiles = []
            for b2 in range(2):
                b = 2 * pair + b2
                rows = rpool.tile([128, NKC * NS], BF16, tag=f"rows{b2}", name=f"rows{b}", bufs=1)
                HNS = NKC * NS // 2
                nc.sync.dma_start(out=rows[:, :HNS], in_=img_d[b, :, :HNS])
                nc.sync.dma_start(out=rows[:, HNS:], in_=img_d[b, :, HNS:])
                tiles.append(rows)
            return tiles

        # ---- resident weights (first few wp chunks land before the big
        # image DMAs so the first patch matmuls can start ASAP) ----
        wp_sb = []

        def emit_wp(kcs):
            for kc in kcs:
                t = wpool.tile([128, C], BF16, tag=f"wp{kc}", name=f"wp{kc}")
                nc.sync.dma_start(out=t[:], in_=wp_d[kc * 128 : (kc + 1) * 128])
                wp_sb.append(t)

        bp_sb = [wpool.tile([128, 1], F32, tag="bp0", name="bp0")]
        nc.sync.dma_start(out=bp_sb[0][:], in_=bp_d[0:128].unsqueeze(1))
        # touch the activation engine once so ACT_TABLE_LOAD happens during
        # the DMA warm-up instead of right before the first feats copy
        actwarm = wpool.tile([128, 1], F32, tag="actwarm", name="actwarm")
        nc.scalar.activation(actwarm[:], bp_sb[0][:], AF.Relu)

        rows_first = emit_rows(0)
        emit_wp(range(NKC))
        for m in range(1, NMC):
            t = wpool.tile([128, 1], F32, tag=f"bp{m}", name=f"bp{m}")
            nc.sync.dma_start(out=t[:], in_=bp_d[m * 128 : (m + 1) * 128].unsqueeze(1))
            bp_sb.append(t)
        b1_sb = []
        for hc in range(NHC):
            t = wpool.tile([128, 1], F32, tag=f"b1{hc}", name=f"b1{hc}")
            nc.sync.dma_start(out=t[:], in_=b1_d[hc * 128 : (hc + 1) * 128].unsqueeze(1))
            b1_sb.append(t)
        w1_sb = {}
        for wname, wd in (("a", w1a_d), ("b", w1b_d)):
            for kc in range(NMC):
                t = wpool.tile([128, HM], BF16, tag=f"w1{wname}{kc}", name=f"w1{wname}{kc}")
                nc.sync.dma_start(out=t[:], in_=wd[kc * 128 : (kc + 1) * 128])
                w1_sb[(wname, kc)] = t
        w2t_sb = wpool.tile([128, NHC * 7], BF16, tag="w2t", name="w2t")
        nc.sync.dma_start(
            out=w2t_sb[:].rearrange("p (hc c) -> p hc c", hc=NHC, c=7),
            in_=w2t_d.rearrange("(hc p) c -> p hc c", hc=NHC),
        )
        # ---- per-batch pipeline stage (finer grain => earlier DVE start,
        # shorter DVE tail) ----
        def stage_a(b, rows):
            """patch embed + a/g + pairwise adds + relu for one batch."""
            b2 = b % 2
            ps_f = [
                psA.tile([128, NS], F32, tag="mm", name=f"psf{b}_{m}")
                for m in range(NMC)
            ]
            for kc in range(NKC):
                for m in range(NMC):
                    nc.tensor.matmul(
                        out=ps_f[m][:],
                        lhsT=wp_sb[kc][:, m * 128 : (m + 1) * 128],
                        rhs=rows[:, kc * NS : (kc + 1) * NS],
                        start=(kc == 0),
                        stop=(kc == NKC - 1),
                    )
            feats = []
            for m in range(NMC):
                ft = fpool.tile(
                    [128, NS], BF16, tag=f"feats{b2}_{m}", name=f"feats{b}_{m}"
                )
                nc.scalar.activation(ft[:], ps_f[m][:], AF.Identity, bias=bp_sb[m][:])
                feats.append(ft)
            ag = {}
            for hc in HC_ORDER:
                for wname, bias in (("a", None), ("b", b1_sb[hc])):
                    ps = psA.tile([128, NS], F32, tag="mm", name=f"ps{wname}{b}_{hc}")
                    for kc in range(NMC):
                        nc.tensor.matmul(
                            out=ps[:],
                            lhsT=w1_sb[(wname, kc)][:, hc * 128 : (hc + 1) * 128],
                            rhs=feats[kc][:],
                            start=(kc == 0),
                            stop=(kc == NMC - 1),
                        )
                    st = agpool.tile(
                        [128, NS], BF16, tag=f"{wname}{b2}_{hc}",
                        name=f"{wname}{b}_{hc}",
                    )
                    if bias is None:
                        nc.scalar.copy(st[:], ps[:])
                    else:
                        nc.scalar.activation(st[:], ps[:], AF.Identity, bias=bias[:])
                    ag[(wname, hc)] = st
            # pairwise adds straight into this batch's half of the s-major
            # pair tile. The broadcast inputs force DVE 1x (one input always
            # has a stride-0 innermost dim), but s-major keeps the pairwise
            # matmul rhs contiguous, which matters more. In-place DVE relu
            # (4x tensor_scalar) follows each half immediately.
            for hc in HC_ORDER:
                half = pair_trelu[hc][:, b2 * S * E : (b2 + 1) * S * E]
                a4 = (
                    ag[("a", hc)][:]
                    .rearrange("p (s i) -> p s i", s=S, i=N)
                    .unsqueeze(2)
                    .broadcast_to([128, S, N, N])
                )
                g4 = (
                    ag[("b", hc)][:]
                    .rearrange("p (s j) -> p s j", s=S, j=N)
                    .unsqueeze(3)
                    .broadcast_to([128, S, N, N])
                )
                o4 = half.rearrange("p (s j i) -> p s j i", s=S, j=N, i=N)
                nc.vector.tensor_tensor(out=o4, in0=a4, in1=g4, op=AX.add)
                nc.vector.tensor_scalar_max(half, half, 0.0)

        def stage_b(pair, trelu):
            """pooled @ W2h via 49 contiguous-rhs N=128 matmuls per HM chunk."""
            ps_o = psB.tile([7, 2 * E], F32, tag="po", name=f"pso{pair}")
            for idx, hc in enumerate(HC_ORDER):
                lhsT = w2t_sb[:, hc * 7 : (hc + 1) * 7]
                t4 = trelu[hc][:].rearrange("p (b2 s e) -> p b2 s e", b2=2, s=S, e=E)
                for s in range(S):
                    nc.tensor.matmul(
                        out=ps_o[:],
                        lhsT=lhsT,
                        rhs=t4[:, :, s, :],
                        start=(idx == 0 and s == 0),
                        stop=(idx == NHC - 1 and s == S - 1),
                    )
            osb = opool.tile([7, 2 * E], F32, tag="osb", name=f"osb{pair}")
            nc.scalar.copy(osb[:], ps_o[:])
            nc.sync.dma_start(out=out_d[pair], in_=osb[:])

        pair_trelu = None
        prev = None  # (pair, trelu)
        rows_next = rows_first
        for pair in range(B_LOC // 2):
            pair_trelu = [
                tpool.tile(
                    [128, 2 * E * S], BF16, tag=f"trelu{hc}", name=f"trelu{pair}_{hc}", bufs=2
                )
                for hc in range(NHC)
            ]
            rows2 = rows_next
            stage_a(2 * pair, rows2[0])
            if pair + 1 < B_LOC // 2:
                rows_next = emit_rows(pair + 1)
            stage_a(2 * pair + 1, rows2[1])
            if prev is not None:
                stage_b(*prev)
            prev = (pair, pair_trelu)
        stage_b(*prev)
    nc.compile()
    return nc


def _prep_weights(W_patch, b_patch, W1a, W1b, b1, W2, b2, Wp, bp, Wpv, bpv, Wr, br, Wrv, brv):
    bf = ml_dtypes.bfloat16
    # wp[k, c] = W_patch[c, ci, ky, kx], k = ci*1024 + ky*32 + kx
    wp = np.ascontiguousarray(
        W_patch.astype(np.float32).transpose(1, 2, 3, 0).reshape(K, C)
    ).astype(bf)
    w_heads = np.concatenate([Wp, Wpv, Wr, Wrv], axis=1).astype(np.float64)  # [256, 7]
    b_heads = np.concatenate([bp, bpv, br, brv]).astype(np.float64)  # [7]
    w2h = (W2.astype(np.float64) @ w_heads / S).astype(np.float32)  # [512, 7]
    bias2h = (b2.astype(np.float64) @ w_heads + b_heads).astype(np.float32)  # [7]
    return {
        "wp": wp,
        "w1a": np.ascontiguousarray(W1a.astype(np.float32)).astype(bf),
        "w1b": np.ascontiguousarray(W1b.astype(np.float32)).astype(bf),
        "w2t": w2h.astype(bf),
        "bpatch": np.ascontiguousarray(b_patch.astype(np.float32)),
        "b1": np.ascontiguousarray(b1.astype(np.float32)),
    }, bias2h


def _im2col(img_bf):
    """[B, N, C_IN, H, W] bf16 -> [B, 128, NKC*NS]; pure permutation (stride-32
    patches don't overlap). dev[b, p, kc*392 + (py*7+px)*8 + n] =
    img[b, n, ci, py*32+ky, px*32+kx] with kc*128+p = ci*1024 + ky*32 + kx.
    s-major columns keep the a/g tiles [p, (s, n)] so the pairwise adds read
    with innermost stride 1 and the pairwise-matmul rhs stays contiguous."""
    x = img_bf.reshape(B, N, C_IN, 7, 32, 7, 32)          # b n ci py ky px kx
    x = x.transpose(0, 2, 4, 6, 3, 5, 1)                  # b ci ky kx py px n
    x = np.ascontiguousarray(x).reshape(B, K, NS)         # b k (s n)
    x = x.reshape(B, NKC, 128, NS).transpose(0, 2, 1, 3)  # b p kc ns
    return np.ascontiguousarray(x).reshape(B, 128, NKC * NS)


def kernel(img_raw, pos, rot, W_patch, b_patch, W1a, W1b, b1, W2, b2,
           Wp, bp, Wpv, bpv, Wr, br, Wrv, brv, _profile=False):
    img_raw = np.asarray(img_raw, dtype=np.float32)
    pos = np.asarray(pos, dtype=np.float32)
    args = [np.asarray(x, dtype=np.float32) for x in
            (W_patch, b_patch, W1a, W1b, b1, W2, b2, Wp, bp, Wpv, bpv, Wr, br, Wrv, brv)]
    weights, bias2h = _prep_weights(*args)

    if "nc" not in _CACHE:
        _CACHE["nc"] = _build_nc()
    nc = _CACHE["nc"]

    img_dev = _im2col(img_raw.astype(ml_dtypes.bfloat16))
    in_maps = []
    for c in range(N_CORES):
        m = dict(weights)
        m["img"] = img_dev[c * B_LOC : (c + 1) * B_LOC].reshape(B_LOC, 128, NKC * NS)
        in_maps.append(m)

    res = run_bass_kernel_spmd(
        nc, in_maps, list(range(N_CORES)), trace=bool(_profile)
    )

    # gather: dev out [B_LOC//2, 7, 128] per core, col = b2*64 + j*8 + i
    heads = np.empty((B, N, N, 7), np.float32)
    for c in range(N_CORES):
        o = res.results[c]["out"]  # [B_LOC//2, 7, 2*E]
        for pair in range(B_LOC // 2):
            for b2 in range(2):
                blk = o[pair][:, b2 * E : (b2 + 1) * E]  # [7, 64]
                # [7, 64] -> [64, 7] -> [j, i, 7] -> [i, j, 7]
                heads[c * B_LOC + pair * 2 + b2] = (
                    blk.T.reshape(N, N, 7).transpose(1, 0, 2)
                )
    heads += bias2h

    pos2 = pos[..., :2]
    d2 = np.sum((pos2[:, :, None, :] - pos2[:, None, :, :]) ** 2, axis=-1)
    mask = (d2 < COMM_RANGE**2) & (~np.eye(N, dtype=bool))
    mf = mask.reshape(B * N * N).astype(np.float32)

    edge_out = heads.reshape(B * N * N, 7) * mf[:, None]
    node_preds = np.zeros((B, N, 1, 60, 60), np.float32)
    if _profile:
        return (edge_out, mask.reshape(B * N * N), node_preds), res
    return edge_out, mask.reshape(B * N * N), node_preds


# revision 32
# speedup vs baseline: 1.0436x; 1.0436x over previous
"""Trainium2 Bass kernel for nn_BEVGNNModelLoaded (gnn_message_passing).

Data-parallel over batch dim B=32 across 8 NeuronCores (4 batches/core).

Math (per batch of N=8 node images):
  feats[n, s, c]   = conv_patch(img[n])            (stride-32 32x32 patches; S=49, C=384)
  a = feats @ W1a; g = feats @ W1b                 ([N, S, HM], HM=512)
  h[i, j, s, :]    = relu(a[i, s] + g[j, s] + b1)
  m[i, j]          = (sum_s h / S) @ W2 + b2       ([N, N, 256])
  heads            = m @ [Wp|Wpv|Wr|Wrv] + biases  ([N, N, 7])
  edge_out         = heads * radius_mask

Device-side structure:
  * Patch conv as matmul over K=3072 (24 chunks of 128). Because stride-32
    patches don't overlap, im2col is a pure permutation — done on the host
    when packing per-core inputs, so each batch's pixels arrive in ONE
    contiguous [128, 9408] DMA.
  * The four heads + W2 collapse: W2h = (W2 @ W_heads)/S, so the device
    computes pooled @ W2h directly ([512, 7]); m is never materialized.
  * The s-pooling folds into the final matmul accumulation (49 strided-rhs
    matmuls per HM chunk) — no reduce instruction at all.
Host side: bias2h add + mask multiply on the [2048, 7] result (trivial),
the radius mask itself, zeros for node_preds.

Compute dtype bf16 (inputs cast on host), fp32 PSUM accumulation.
"""
import numpy as np
import ml_dtypes
from contextlib import ExitStack

import concourse.bass as bass
import concourse.bacc as bacc
import concourse.tile as tile
from concourse import mybir
from concourse.bass_utils import run_bass_kernel_spmd

F32 = mybir.dt.float32
BF16 = mybir.dt.bfloat16
AX = mybir.AluOpType
AF = mybir.ActivationFunctionType

# problem constants (hardcoded per contract)
B, N, C_IN, H, W = 32, 8, 3, 224, 224
PATCH, C, S, HM, CO = 32, 384, 49, 512, 256
COMM_RANGE = 0.5
N_CORES = 8
B_LOC = B // N_CORES          # 4 batches per core
K = C_IN * PATCH * PATCH      # 3072 contraction
NKC = K // 128                # 24 K-chunks
NS = N * S                    # 392 free cols per batch
E = N * N                     # 64 edges per batch
NHC = HM // 128               # 4 HM chunks
NMC = C // 128                # 3 C chunks

HC_ORDER = (2, 3, 0, 1)   # ACT-relu'd chunks first, DVE-relu'd last

_CACHE = {}


def _build_nc():
    nc = bacc.Bacc(trn_type="TRN2", target_bir_lowering=False, debug=False)
    # img already im2col'd on host: [B_LOC, 128, NKC*NS]; [p, kc*392 + s*8 + n]
    img_d = nc.dram_tensor("img", [B_LOC, 128, NKC * NS], BF16, kind="ExternalInput").ap()
    wp_d = nc.dram_tensor("wp", [K, C], BF16, kind="ExternalInput").ap()
    w1a_d = nc.dram_tensor("w1a", [C, HM], BF16, kind="ExternalInput").ap()
    w1b_d = nc.dram_tensor("w1b", [C, HM], BF16, kind="ExternalInput").ap()
    w2t_d = nc.dram_tensor("w2t", [HM, 7], BF16, kind="ExternalInput").ap()
    bp_d = nc.dram_tensor("bpatch", [C], F32, kind="ExternalInput").ap()
    b1_d = nc.dram_tensor("b1", [HM], F32, kind="ExternalInput").ap()
    out_d = nc.dram_tensor("out", [B_LOC // 2, 7, 2 * E], F32, kind="ExternalOutput").ap()

    with tile.TileContext(nc) as tc, ExitStack() as ctx:
        wpool = ctx.enter_context(tc.tile_pool(name="wpool", bufs=1))
        rpool = ctx.enter_context(tc.tile_pool(name="rpool", bufs=2))
        fpool = ctx.enter_context(tc.tile_pool(name="fpool", bufs=2))
        agpool = ctx.enter_context(tc.tile_pool(name="agpool", bufs=2))
        tpool = ctx.enter_context(tc.tile_pool(name="tpool", bufs=2))
        opool = ctx.enter_context(tc.tile_pool(name="opool", bufs=2))
        psA = ctx.enter_context(tc.tile_pool(name="psA", bufs=6, space="PSUM"))
        psB = ctx.enter_context(tc.tile_pool(name="psB", bufs=2, space="PSUM"))

        # first pair's image DMAs go out BEFORE the weight DMAs so the PE
        # isn't stuck behind ~5MB of weight traffic at kernel start
        def emit_rows(pair):
            tiles = []
            for b2 in range(2):
                b = 2 * pair + b2
                rows = rpool.tile([128, NKC * NS], BF16, tag=f"rows{b2}", name=f"rows{b}", bufs=1)
                nc.sync.dma_start(out=rows[:], in_=img_d[b])
                tiles.append(rows)
            return tiles

        # ---- resident weights (first few wp chunks land before the big
        # image DMAs so the first patch matmuls can start ASAP) ----
        wp_sb = []

        def emit_wp(kcs):
            for kc in kcs:
                t = wpool.tile([128, C], BF16, tag=f"wp{kc}", name=f"wp{kc}")
                nc.sync.dma_start(out=t[:], in_=wp_d[kc * 128 : (kc + 1) * 128])
                wp_sb.append(t)

        bp_sb = [wpool.tile([128, 1], F32, tag="bp0", name="bp0")]
        nc.sync.dma_start(out=bp_sb[0][:], in_=bp_d[0:128].unsqueeze(1))
        # touch the activation engine once so ACT_TABLE_LOAD happens during
        # the DMA warm-up instead of right before the first feats copy
        actwarm = wpool.tile([128, 1], F32, tag="actwarm", name="actwarm")
        nc.scalar.activation(actwarm[:], bp_sb[0][:], AF.Relu)

        rows_first = emit_rows(0)
        emit_wp(range(NKC))
        for m in range(1, NMC):
            t = wpool.tile([128, 1], F32, tag=f"bp{m}", name=f"bp{m}")
            nc.sync.dma_start(out=t[:], in_=bp_d[m * 128 : (m + 1) * 128].unsqueeze(1))
            bp_sb.append(t)
        b1_sb = []
        for hc in range(NHC):
            t = wpool.tile([128, 1], F32, tag=f"b1{hc}", name=f"b1{hc}")
            nc.sync.dma_start(out=t[:], in_=b1_d[hc * 128 : (hc + 1) * 128].unsqueeze(1))
            b1_sb.append(t)
        w1_sb = {}
        for wname, wd in (("a", w1a_d), ("b", w1b_d)):
            for kc in range(NMC):
                t = wpool.tile([128, HM], BF16, tag=f"w1{wname}{kc}", name=f"w1{wname}{kc}")
                nc.sync.dma_start(out=t[:], in_=wd[kc * 128 : (kc + 1) * 128])
                w1_sb[(wname, kc)] = t
        w2t_sb = wpool.tile([128, NHC * 7], BF16, tag="w2t", name="w2t")
        nc.sync.dma_start(
            out=w2t_sb[:].rearrange("p (hc c) -> p hc c", hc=NHC, c=7),
            in_=w2t_d.rearrange("(hc p) c -> p hc c", hc=NHC),
        )
        # ---- per-batch pipeline stage (finer grain => earlier DVE start,
        # shorter DVE tail) ----
        def stage_a(b, rows):
            """patch embed + a/g + pairwise adds + relu for one batch."""
            b2 = b % 2
            ps_f = [
                psA.tile([128, NS], F32, tag="mm", name=f"psf{b}_{m}")
                for m in range(NMC)
            ]
            for kc in range(NKC):
                for m in range(NMC):
                    nc.tensor.matmul(
                        out=ps_f[m][:],
                        lhsT=wp_sb[kc][:, m * 128 : (m + 1) * 128],
                        rhs=rows[:, kc * NS : (kc + 1) * NS],
                        start=(kc == 0),
                        stop=(kc == NKC - 1),
                    )
            feats = []
            for m in range(NMC):
                ft = fpool.tile(
                    [128, NS], BF16, tag=f"feats{b2}_{m}", name=f"feats{b}_{m}"
                )
                nc.scalar.activation(ft[:], ps_f[m][:], AF.Identity, bias=bp_sb[m][:])
                feats.append(ft)
            ag = {}
            for hc in HC_ORDER:
                for wname, bias in (("a", None), ("b", b1_sb[hc])):
                    ps = psA.tile([128, NS], F32, tag="mm", name=f"ps{wname}{b}_{hc}")
                    for kc in range(NMC):
                        nc.tensor.matmul(
                            out=ps[:],
                            lhsT=w1_sb[(wname, kc)][:, hc * 128 : (hc + 1) * 128],
                            rhs=feats[kc][:],
                            start=(kc == 0),
                            stop=(kc == NMC - 1),
                        )
                    st = agpool.tile(
                        [128, NS], BF16, tag=f"{wname}{b2}_{hc}",
                        name=f"{wname}{b}_{hc}",
                    )
                    if bias is None:
                        nc.scalar.copy(st[:], ps[:])
                    else:
                        nc.scalar.activation(st[:], ps[:], AF.Identity, bias=bias[:])
                    ag[(wname, hc)] = st
            # pairwise adds straight into this batch's half of the s-major
            # pair tile. The broadcast inputs force DVE 1x (one input always
            # has a stride-0 innermost dim), but s-major keeps the pairwise
            # matmul rhs contiguous, which matters more. In-place DVE relu
            # (4x tensor_scalar) follows each half immediately.
            for hc in HC_ORDER:
                half = pair_trelu[hc][:, b2 * S * E : (b2 + 1) * S * E]
                a4 = (
                    ag[("a", hc)][:]
                    .rearrange("p (s i) -> p s i", s=S, i=N)
                    .unsqueeze(2)
                    .broadcast_to([128, S, N, N])
                )
                g4 = (
                    ag[("b", hc)][:]
                    .rearrange("p (s j) -> p s j", s=S, j=N)
                    .unsqueeze(3)
                    .broadcast_to([128, S, N, N])
                )
                o4 = half.rearrange("p (s j i) -> p s j i", s=S, j=N, i=N)
                nc.vector.tensor_tensor(out=o4, in0=a4, in1=g4, op=AX.add)
                nc.vector.tensor_scalar_max(half, half, 0.0)

        def stage_b(pair, trelu):
            """pooled @ W2h via 49 contiguous-rhs N=128 matmuls per HM chunk."""
            ps_o = psB.tile([7, 2 * E], F32, tag="po", name=f"pso{pair}")
            for idx, hc in enumerate(HC_ORDER):
                lhsT = w2t_sb[:, hc * 7 : (hc + 1) * 7]
                t4 = trelu[hc][:].rearrange("p (b2 s e) -> p b2 s e", b2=2, s=S, e=E)
                for s in range(S):
                    nc.tensor.matmul(
                        out=ps_o[:],
                        lhsT=lhsT,
                        rhs=t4[:, :, s, :],
                        start=(idx == 0 and s == 0),
                        stop=(idx == NHC - 1 and s == S - 1),
                    )
            osb = opool.tile([7, 2 * E], F32, tag="osb", name=f"osb{pair}")
            nc.scalar.copy(osb[:], ps_o[:])
            nc.sync.dma_start(out=out_d[pair], in_=osb[:])

        pair_trelu = None
        prev = None  # (pair, trelu)
        rows_next = rows_first
        for pair in range(B_LOC // 2):
            pair_trelu = [
                tpool.tile(
                    [128, 2 * E * S], BF16, tag=f"trelu{hc}", name=f"trelu{pair}_{hc}", bufs=2
                )
                for hc in range(NHC)
            ]
            rows2 = rows_next
            stage_a(2 * pair, rows2[0])
            if pair + 1 < B_LOC // 2:
                rows_next = emit_rows(pair + 1)
            stage_a(2 * pair + 1, rows2[1])
            if prev is not None:
                stage_b(*prev)
            prev = (pair, pair_trelu)
        stage_b(*prev)
    nc.compile()
    return nc


def _prep_weights(W_patch, b_patch, W1a, W1b, b1, W2, b2, Wp, bp, Wpv, bpv, Wr, br, Wrv, brv):
    bf = ml_dtypes.bfloat16
    # wp[k, c] = W_patch[c, ci, ky, kx], k = ci*1024 + ky*32 + kx
    wp = np.ascontiguousarray(
        W_patch.astype(np.float32).transpose(1, 2, 3, 0).reshape(K, C)
    ).astype(bf)
    w_heads = np.concatenate([Wp, Wpv, Wr, Wrv], axis=1).astype(np.float64)  # [256, 7]
    b_heads = np.concatenate([bp, bpv, br, brv]).astype(np.float64)  # [7]
    w2h = (W2.astype(np.float64) @ w_heads / S).astype(np.float32)  # [512, 7]
    bias2h = (b2.astype(np.float64) @ w_heads + b_heads).astype(np.float32)  # [7]
    return {
        "wp": wp,
        "w1a": np.ascontiguousarray(W1a.astype(np.float32)).astype(bf),
        "w1b": np.ascontiguousarray(W1b.astype(np.float32)).astype(bf),
        "w2t": w2h.astype(bf),
        "bpatch": np.ascontiguousarray(b_patch.astype(np.float32)),
        "b1": np.ascontiguousarray(b1.astype(np.float32)),
    }, bias2h


def _im2col(img_bf):
    """[B, N, C_IN, H, W] bf16 -> [B, 128, NKC*NS]; pure permutation (stride-32
    patches don't overlap). dev[b, p, kc*392 + (py*7+px)*8 + n] =
    img[b, n, ci, py*32+ky, px*32+kx] with kc*128+p = ci*1024 + ky*32 + kx.
    s-major columns keep the a/g tiles [p, (s, n)] so the pairwise adds read
    with innermost stride 1 and the pairwise-matmul rhs stays contiguous."""
    x = img_bf.reshape(B, N, C_IN, 7, 32, 7, 32)          # b n ci py ky px kx
    x = x.transpose(0, 2, 4, 6, 3, 5, 1)                  # b ci ky kx py px n
    x = np.ascontiguousarray(x).reshape(B, K, NS)         # b k (s n)
    x = x.reshape(B, NKC, 128, NS).transpose(0, 2, 1, 3)  # b p kc ns
    return np.ascontiguousarray(x).reshape(B, 128, NKC * NS)


def kernel(img_raw, pos, rot, W_patch, b_patch, W1a, W1b, b1, W2, b2,
           Wp, bp, Wpv, bpv, Wr, br, Wrv, brv, _profile=False):
    img_raw = np.asarray(img_raw, dtype=np.float32)
    pos = np.asarray(pos, dtype=np.float32)
    args = [np.asarray(x, dtype=np.float32) for x in
            (W_patch, b_patch, W1a, W1b, b1, W2, b2, Wp, bp, Wpv, bpv, Wr, br, Wrv, brv)]
    weights, bias2h = _prep_weights(*args)

    if "nc" not in _CACHE:
        _CACHE["nc"] = _build_nc()
    nc = _CACHE["nc"]

    img_dev = _im2col(img_raw.astype(ml_dtypes.bfloat16))
    in_maps = []
    for c in range(N_CORES):
        m = dict(weights)
        m["img"] = img_dev[c * B_LOC : (c + 1) * B_LOC].reshape(B_LOC, 128, NKC * NS)
        in_maps.append(m)

    res = run_bass_kernel_spmd(
        nc, in_maps, list(range(N_CORES)), trace=bool(_profile)
    )

    # gather: dev out [B_LOC//2, 7, 128] per core, col = b2*64 + j*8 + i
    heads = np.empty((B, N, N, 7), np.float32)
    for c in range(N_CORES):
        o = res.results[c]["out"]  # [B_LOC//2, 7, 2*E]
        for pair in range(B_LOC // 2):
            for b2 in range(2):
                blk = o[pair][:, b2 * E : (b2 + 1) * E]  # [7, 64]
                # [7, 64] -> [64, 7] -> [j, i, 7] -> [i, j, 7]
                heads[c * B_LOC + pair * 2 + b2] = (
                    blk.T.reshape(N, N, 7).transpose(1, 0, 2)
                )
    heads += bias2h

    pos2 = pos[..., :2]
    d2 = np.sum((pos2[:, :, None, :] - pos2[:, None, :, :]) ** 2, axis=-1)
    mask = (d2 < COMM_RANGE**2) & (~np.eye(N, dtype=bool))
    mf = mask.reshape(B * N * N).astype(np.float32)

    edge_out = heads.reshape(B * N * N, 7) * mf[:, None]
    node_preds = np.zeros((B, N, 1, 60, 60), np.float32)
    if _profile:
        return (edge_out, mask.reshape(B * N * N), node_preds), res
    return edge_out, mask.reshape(B * N * N), node_preds
